# revision 1
# baseline (speedup 1.0000x reference)
"""GATv2 3-layer GNN encoder on 8 TRN2 NeuronCores (Bass/Tile).

Sharding: nodes split into 8 contiguous shards (graph-parallel by dst).
Each core owns the edges into its shard; segment-softmax + scatter-add
become per-core one-hot ("staircase") matmuls accumulated in PSUM over
125-node blocks. Self-loops (with host-precomputed mean edge_attr) are
packed as ordinary edges. Gather tables use a parity layout ([r>>1,
r&1, :]) so indices fit int16 with a strided (elem_step) dma_gather and
no >32k-row copies. x is replicated so layer 1 builds its full gather
table locally (split-float bf16 K=25 matmuls reproduce the f32 dense
exactly); layers 2/3 AllGather their tables, with the next layer's Wl
dense interleaved into the previous LN/ELU pass so the collective
launches immediately and the Wr pass runs under it. Layernorm stats use
a tiny AllReduce; pooling reduces an AllGather'd transposed h3 in two
halves, the first launched mid-pass so it overlaps. Segment-max is
skipped: logits of this model are bounded (|logit| < ~20) and softmax
is shift-invariant, so exp() without max-shift is numerically safe.

Per-instruction overheads dominate this part (each dma_start occupies
the shared HWDGE descriptor generator ~0.6us), so all hot loads are
batched per 125-node block with multi-dim access patterns, index tables
stay SBUF-resident, and the per-edge softmax math uses per-head
scalar_tensor_tensor-with-accum / tensor_scalar forms that keep logits
in f32 and hit the DVE fast path. (Note: tensor_tensor_reduce compiles
but hangs the DVE at runtime here; scalar_tensor_tensor is the safe
fused form.)
"""

import numpy as np
import ml_dtypes

import concourse.mybir as mybir
from concourse.bacc import Bacc
from concourse.tile import TileContext
from concourse.bass_utils import run_bass_kernel_spmd

F32 = mybir.dt.float32
BF16 = mybir.dt.bfloat16
I16 = mybir.dt.int16
AF = mybir.ActivationFunctionType
ALU = mybir.AluOpType
AX = mybir.AxisListType

NCORE = 8
C = 128
LAYERS = [(8, 4), (512, 2), (256, 1)]
BLK = 125
LO_LIMIT = 32768
HALF = 3200  # local-row split: class A rows [0, HALF), class B [HALF, tpad)

bf = ml_dtypes.bfloat16


def _wrap_idx(a):
    a = np.asarray(a, np.int16)
    assert len(a) % 16 == 0
    w = np.ascontiguousarray(a.reshape(-1, 16).T)
    return np.tile(w, (8, 1))


def _rep(v, rows=128):
    return np.tile(np.asarray(v, np.float32).reshape(1, -1), (rows, 1))


def _splitx(xT):
    """[8, n] f32 -> [25, n] bf16 split-float stack: rows 0:8 x_hi, 8:16
    x_hi again, 16:24 x_lo, 24 ones. Paired with _splitw so a single K=25
    bf16 matmul reproduces the f32 product to ~2^-16."""
    hi = xT.astype(bf)
    lo = (xT - hi.astype(np.float32)).astype(bf)
    ones = np.ones((1, xT.shape[1]), bf)
    return np.ascontiguousarray(np.concatenate([hi, hi, lo, ones], axis=0))


def _splitw(W, bias):
    """[8, HC] f32 weights + [HC] bias -> [25, HC] bf16: W_hi, W_lo, W_hi,
    bias."""
    hi = W.astype(bf)
    lo = (W - hi.astype(np.float32)).astype(bf)
    return np.ascontiguousarray(np.concatenate(
        [hi, lo, hi, np.asarray(bias, np.float32).reshape(1, -1).astype(bf)], axis=0))


def _preprocess(x, edge_index, edge_attr, batch, G):
    N = x.shape[0]
    shard = N // NCORE
    assert shard * NCORE == N and shard % BLK == 0
    nblk = shard // BLK
    tpad = ((shard + 127) // 128) * 128

    src0 = edge_index[0].astype(np.int64)
    dst0 = edge_index[1].astype(np.int64)

    # self-loop edge_attr = mean of attrs of edges entering the node
    cnt_all = np.zeros(N, np.float64)
    np.add.at(cnt_all, dst0, 1.0)
    la_sum = np.zeros((N, 3), np.float64)
    np.add.at(la_sum, dst0, edge_attr.astype(np.float64))
    loop_attr = (la_sum / np.maximum(cnt_all, 1.0)[:, None]).astype(np.float32)

    ar = np.arange(N, dtype=np.int64)
    src = np.concatenate([src0, ar])
    dst = np.concatenate([dst0, ar])
    ea2 = np.concatenate([edge_attr.astype(np.float32), loop_attr], axis=0)

    core_of = dst // shard
    grow = (src // shard) * tpad + (src % shard)
    gidx = (grow >> 1).astype(np.int16)  # parity-split row index, < 32768
    inB = (grow & 1).astype(bool)

    per_core_edges = []
    for k in range(NCORE):
        sel = np.nonzero(core_of == k)[0]
        dl = dst[sel] - k * shard
        blk = dl // BLK
        eb = inB[sel]
        blocks = []
        for b in range(nblk):
            m = blk == b
            blocks.append((sel[m & ~eb], sel[m & eb]))
        per_core_edges.append(blocks)

    c_lo = [max(-(-len(per_core_edges[k][b][0]) // 128) for k in range(NCORE))
            for b in range(nblk)]
    c_hi = [max(-(-len(per_core_edges[k][b][1]) // 128) for k in range(NCORE))
            for b in range(nblk)]
    tot_chunks = sum(c_lo) + sum(c_hi)

    xF = np.zeros((8, NCORE * tpad), np.float32)
    for k in range(NCORE):
        xF[:, k * tpad: k * tpad + shard] = x[k * shard:(k + 1) * shard].T
    meta = dict(N=N, G=G, shard=shard, nblk=nblk, tpad=tpad,
                c_lo=c_lo, c_hi=c_hi, tot_chunks=tot_chunks, xF=_splitx(xF))

    per_core = []
    for k in range(NCORE):
        P_pack = np.zeros((128, tot_chunks, 128), bf)
        W_pack = np.zeros((128, tot_chunks, 128), bf)
        idx_lo_parts, idx_hi_parts = [], []
        cpos = 0
        for b in range(nblk):
            e_lo, e_hi = per_core_edges[k][b]
            for kind, edges, cnt in (("lo", e_lo, c_lo[b]), ("hi", e_hi, c_hi[b])):
                if cnt == 0:
                    continue
                nslot = cnt * 128
                rows = np.zeros(nslot, np.int16)
                ne = len(edges)
                if ne:
                    rows[:ne] = gidx[edges]
                (idx_lo_parts if kind == "lo" else idx_hi_parts).append(rows)
                for c in range(cnt):
                    e_ids = edges[c * 128: c * 128 + 128]
                    nv = len(e_ids)
                    P = np.zeros((128, 128), np.float32)
                    if nv:
                        dr = (dst[e_ids] - k * shard) - b * BLK
                        P[np.arange(nv), dr] = 1.0
                        W_pack[125:128, cpos, 0:nv] = ea2[e_ids].T.astype(bf)
                    P_pack[:, cpos, :] = P.astype(bf)
                    W_pack[0:125, cpos, :] = P.T[0:125].astype(bf)
                    cpos += 1
        assert cpos == tot_chunks
        xT = np.zeros((8, tpad), np.float32)
        xT[:, :shard] = x[k * shard: (k + 1) * shard].T
        xT = _splitx(xT)
        per_core.append(dict(
            P_pack=np.ascontiguousarray(P_pack.reshape(128, -1)),
            W_pack=np.ascontiguousarray(W_pack.reshape(128, -1)),
            idx_lo=_wrap_idx(np.concatenate(idx_lo_parts)) if idx_lo_parts else np.zeros((128, 8), np.int16),
            idx_hi=_wrap_idx(np.concatenate(idx_hi_parts)) if idx_hi_parts else np.zeros((128, 8), np.int16),
            xT=xT,
        ))

    # pooling pieces: (graph, bank core, col lo, col hi) — global/static
    pieces = []
    bt = batch.astype(np.int64)
    starts = np.searchsorted(bt, np.arange(G))
    ends = np.searchsorted(bt, np.arange(G), side="right")
    for g in range(G):
        s, e = int(starts[g]), int(ends[g])
        for k in range(NCORE):
            a = max(s, k * shard) - k * shard
            b_ = min(e, (k + 1) * shard) - k * shard
            if b_ > a:
                pieces.append((g, k, a, b_))
    gcnt = (ends - starts).astype(np.float64)
    ginv = (1.0 / np.maximum(gcnt, 1.0)).astype(np.float32)
    gmask = (gcnt > 0).astype(np.float32)
    return meta, per_core, pieces, ginv, gmask


def _build(meta, params, pieces, ginv, gmask):
    N = meta["N"]; G = meta["G"]; shard = meta["shard"]
    nblk = meta["nblk"]; tpad = meta["tpad"]
    c_lo = meta["c_lo"]; c_hi = meta["c_hi"]; tot_chunks = meta["tot_chunks"]
    GP = ((G + 63) // 64) * 64

    nc = Bacc()
    shared = {}

    def inp(name, arr):
        arr = np.ascontiguousarray(arr)
        t = nc.declare_dram_parameter(name, list(arr.shape), mybir.dt.from_np(arr.dtype), isOutput=False)
        shared[name] = arr
        return t

    def pinp(name, shape, npdt):
        return nc.declare_dram_parameter(name, list(shape), mybir.dt.from_np(np.dtype(npdt)), isOutput=False)

    P_t = pinp("P_pack", (128, tot_chunks * 128), bf)
    W_t = pinp("W_pack", (128, tot_chunks * 128), bf)
    nlo = max(8, 128 * sum(c_lo) // 16)
    nhi = max(8, 128 * sum(c_hi) // 16)
    ilo_t = pinp("idx_lo", (128, nlo), np.int16)
    ihi_t = pinp("idx_hi", (128, nhi), np.int16)
    xT_t = pinp("xT", (25, tpad), bf)
    xF_t = nc.declare_dram_parameter("xF", [25, NCORE * tpad], BF16, isOutput=False)
    shared["xF"] = np.ascontiguousarray(meta["xF"])

    id128b = inp("id128b", np.eye(128, dtype=bf))
    zbf_t = inp("zbf", np.zeros((128, 128), bf))
    id128f = inp("id128f", np.eye(128, dtype=np.float32))
    ginv_t = inp("ginv", np.pad(ginv, (0, GP - G)).reshape(-1, 1))
    gmask_t = inp("gmask", np.pad(gmask, (0, GP - G)).reshape(-1, 1))

    L = []
    for li, (din, H) in enumerate(LAYERS, 1):
        HC = H * C
        d = dict(H=H, HC=HC, din=din)
        if din <= 8:
            d["W25l"] = inp(f"W25l{li}", _splitw(params[f"Wl{li}"], np.zeros(HC, np.float32)))
            d["W25r"] = inp(f"W25r{li}", _splitw(params[f"Wr{li}"], params[f"br{li}"] + params[f"bl{li}"]))
        else:
            d["Wl"] = inp(f"Wl{li}", params[f"Wl{li}"].astype(bf))
            d["Wr"] = inp(f"Wr{li}", params[f"Wr{li}"].astype(bf))
            d["brbl_rep"] = inp(f"brbl{li}", _rep(params[f"br{li}"] + params[f"bl{li}"]))
        d["We"] = inp(f"Web{li}", params[f"We{li}"].astype(bf))
        d["att_rep"] = inp(f"attrep{li}", _rep(params[f"att{li}"].reshape(-1)).astype(bf))
        d["bobl_rep"] = inp(f"bobl{li}", _rep(params[f"bo{li}"] + params[f"bl{li}"]))
        d["lnw_rep"] = inp(f"lnwr{li}", _rep(params[f"lnw{li}"]))
        d["lnb_rep"] = inp(f"lnbr{li}", _rep(params[f"lnb{li}"]))
        d["inv_kn"] = 1.0 / (N * HC)
        L.append(d)

    y_out = nc.declare_dram_parameter("y", [G, 2 * C], F32, isOutput=True)

    HCm = max(d["HC"] for d in L)
    # parity view: row r of the gather table lives at [r>>1, r&1, :]
    xl_tbl = [nc.dram_tensor(f"xltbl{i}", [NCORE * tpad // 2, 2, d["HC"]], BF16,
                             addr_space="Local" if i == 0 else "Shared")
              for i, d in enumerate(L)]
    ag_in = [None] + [nc.dram_tensor(f"agin{i}", [tpad, d["HC"]], BF16)
                      for i, d in enumerate(L) if i > 0]
    xr_tbl = [nc.dram_tensor(f"xrtbl{i}", [tpad, d["HC"]], BF16) for i, d in enumerate(L)]
    hT_tbl = [nc.dram_tensor(f"hT{i}", [d["HC"], tpad], BF16) for i, d in enumerate(L)]
    op_tbl = nc.dram_tensor("outpre", [tpad, HCm], BF16)
    st_in = nc.dram_tensor("stin", [1, 2], F32)
    st_out = nc.dram_tensor("stout", [1, 2], F32, addr_space="Shared")
    # pooling input split in halves so the first AllGather overlaps pass B
    HB = (nblk // 2) * BLK
    hT3h = [nc.dram_tensor("hT3a", [C, HB], BF16),
            nc.dram_tensor("hT3b", [C, tpad - HB], BF16)]
    h3T_agh = [nc.dram_tensor("h3Taga", [NCORE * C, HB], BF16, addr_space="Shared"),
               nc.dram_tensor("h3Tagb", [NCORE * C, tpad - HB], BF16, addr_space="Shared")]
    RG = [list(range(NCORE))]
    GRP = 7
    ntile = tpad // 128
    assert ntile % GRP == 0

    with TileContext(nc, num_cores=NCORE) as tc:
        with tc.tile_pool(name="const", bufs=1) as cpool, \
             tc.tile_pool(name="work", bufs=2) as pool, \
             tc.tile_pool(name="gat", bufs=2) as gpool, \
             tc.tile_pool(name="persist", bufs=1) as apool, \
             tc.tile_pool(name="ps", bufs=2, space="PSUM") as pp, \
             tc.tile_pool(name="psa", bufs=1, space="PSUM") as ppa:

            idb = cpool.tile([128, 128], BF16)
            nc.sync.dma_start(out=idb[:], in_=id128b[:])
            idf = cpool.tile([128, 128], F32)
            nc.sync.dma_start(out=idf[:], in_=id128f[:])
            ilo = cpool.tile([128, nlo], I16)
            nc.sync.dma_start(out=ilo[:], in_=ilo_t[:])
            ihi = cpool.tile([128, nhi], I16)
            nc.sync.dma_start(out=ihi[:], in_=ihi_t[:])

            def flush_dense(ob, dsts):
                """Store staged ob[:, 0:len(dsts), :] to (tensor,row) dsts,
                merging row-consecutive runs into single DMAs."""
                i = 0
                while i < len(dsts):
                    t0, r0 = dsts[i]
                    j = i
                    while (j + 1 < len(dsts) and dsts[j + 1][0] is t0
                           and dsts[j + 1][1] == dsts[j][1] + 128):
                        j += 1
                    cnt = j - i + 1
                    if len(t0.shape) == 3:  # parity-layout gather table
                        dst = (t0[r0 // 2:(r0 + cnt * 128) // 2, :, :]
                               .rearrange("r two c -> (r two) c"))
                    else:
                        dst = t0[r0: r0 + cnt * 128, :]
                    nc.sync.dma_start(out=dst.rearrange("(t p) c -> p t c", t=cnt),
                                      in_=ob[:, i:i + cnt, :])
                    i = j + 1

            class DenseEmitter:
                """Stage dense-tile outputs in GRP groups across calls."""

                def __init__(self, li, d, wsel, dst_fn, brt=None, xsrc=None):
                    self.li, self.d, self.wsel = li, d, wsel
                    self.dst_fn, self.brt, self.xsrc = dst_fn, brt, xsrc
                    self.ob = None
                    self.dsts = []

                def emit(self, n):
                    d = self.d
                    HC, din = d["HC"], d["din"]
                    kt = din // 128
                    if self.ob is None:
                        self.ob = pool.tile([128, GRP, HC], BF16, tag="dob", bufs=4)
                        self.dsts = []
                    psd = pp.tile([128, HC], F32, tag="big")
                    if din <= 8:
                        nc.tensor.matmul(psd[:], self.xsrc(n), self.wsel, start=True, stop=True)
                    else:
                        gm = n % ntile
                        lhsT = pool.tile([128, kt, 128], BF16, tag="dh", bufs=4)
                        nc.sync.dma_start(
                            out=lhsT[:],
                            in_=hT_tbl[self.li - 1][:, gm * 128:(gm + 1) * 128]
                            .rearrange("(k p) t -> p k t", k=kt))
                        for k in range(kt):
                            nc.tensor.matmul(psd[:], lhsT[:, k, :], self.wsel[:, k, 0:HC],
                                             start=(k == 0), stop=(k == kt - 1))
                    j = len(self.dsts)
                    if self.brt is not None:
                        nc.vector.tensor_tensor(out=self.ob[:, j, :], in0=psd[:],
                                                in1=self.brt[0:128, 0:HC], op=ALU.add)
                    elif n % 2 == 0:
                        nc.scalar.copy(out=self.ob[:, j, :], in_=psd[:])
                    else:
                        nc.vector.tensor_scalar(out=self.ob[:, j, :], in0=psd[:], scalar1=1.0,
                                                scalar2=None, op0=ALU.mult)
                    self.dsts.append(self.dst_fn(n))
                    if len(self.dsts) == GRP:
                        flush_dense(self.ob, self.dsts)
                        self.ob = None

                def flush(self):
                    if self.ob is not None:
                        flush_dense(self.ob, self.dsts)
                        self.ob = None

            # ---------------- layer 1 dense-all (x replicated, no AG) ------
            d = L[0]
            HC1 = d["HC"]
            w25l = apool.tile([25, HCm], BF16, tag="w25l")
            nc.sync.dma_start(out=w25l[:, 0:HC1], in_=d["W25l"][:])
            w25r = apool.tile([25, HCm], BF16, tag="w25r")
            nc.sync.dma_start(out=w25r[:, 0:HC1], in_=d["W25r"][:])
            for wsel, src_t, ncols, dst_fn in (
                    (w25l[:, 0:HC1], xF_t, NCORE * ntile,
                     lambda n: (xl_tbl[0], n * 128)),
                    (w25r[:, 0:HC1], xT_t, ntile,
                     lambda n: (xr_tbl[0], n * 128))):
                for n0 in range(0, ncols, GRP):
                    xFg = pool.tile([25, GRP * 128], BF16, tag="xg", bufs=4)
                    nc.sync.dma_start(out=xFg[:], in_=src_t[:, n0 * 128:(n0 + GRP) * 128])
                    em = DenseEmitter(0, d, wsel, dst_fn,
                                      xsrc=lambda n: xFg[:, (n % GRP) * 128:(n % GRP + 1) * 128])
                    for n in range(n0, n0 + GRP):
                        em.emit(n)
                    em.flush()

            # ---------------- layers ----------------
            for li, d in enumerate(L):
                H, HC, din = d["H"], d["HC"], d["din"]

                attb = apool.tile([128, HCm], BF16, tag="attb")
                nc.sync.dma_start(out=attb[0:128, 0:HC], in_=d["att_rep"][:])
                statsum = apool.tile([128, 2], F32, tag="stats")
                nc.vector.memset(statsum[:], 0.0)
                bob = apool.tile([128, HCm], F32, tag="bob")
                nc.sync.dma_start(out=bob[0:128, 0:HC], in_=d["bobl_rep"][:])

                xl_ev = xl_tbl[li][:, 0, :]
                xl_od = xl_tbl[li][:, 1, :]
                cpos = 0
                lo_off = 0
                hi_off = 0
                for b in range(nblk):
                    ncl, nchh = c_lo[b], c_hi[b]
                    nch = ncl + nchh
                    rw = pool.tile([128, HC], BF16, tag="rw", bufs=3)
                    nc.sync.dma_start(out=rw[0:125, :], in_=xr_tbl[li][b * BLK:b * BLK + 125, :])
                    nc.sync.dma_start(out=rw[125:128, :], in_=d["We"][:])
                    wcb = pool.tile([128, nch, 128], BF16, tag="wcb", bufs=3)
                    nc.sync.dma_start(out=wcb[:], in_=W_t[:, cpos * 128:(cpos + nch) * 128]
                                      .rearrange("p (n t) -> p n t", n=nch))
                    pcb = pool.tile([128, nch, 128], BF16, tag="pcb", bufs=3)
                    nc.sync.dma_start(out=pcb[:], in_=P_t[:, cpos * 128:(cpos + nch) * 128]
                                      .rearrange("p (n t) -> p n t", n=nch))
                    gt = gpool.tile([128, nch * HC], BF16, tag="gt")
                    GSTEP = 6
                    for ncc, base, itbl, off, xlv in (
                            (ncl, 0, ilo, lo_off, xl_ev),
                            (nchh, ncl, ihi, hi_off, xl_od)):
                        for g0 in range(0, ncc, GSTEP):
                            gn = min(GSTEP, ncc - g0)
                            o16 = (off + g0 * 128) // 16
                            nc.gpsimd.dma_gather(
                                out_ap=gt[:, (base + g0) * HC:(base + g0 + gn) * HC]
                                .rearrange("p (n c) -> p n c", n=gn),
                                in_ap=xlv, idxs_ap=itbl[:, o16:o16 + gn * 8],
                                num_idxs=gn * 128, num_idxs_reg=gn * 128,
                                elem_size=HC, elem_step=2 * HC)
                    lo_off += ncl * 128
                    hi_off += nchh * 128

                    out_ps = ppa.tile([125, HC], F32, tag="acc")
                    den_ps = ppa.tile([125, max(H, 4)], F32, tag="den")
                    lgB = pool.tile([128, nch * H], F32, tag="lgB")
                    for c in range(nch):
                        gsl = gt[:, c * HC:(c + 1) * HC]
                        zps = pp.tile([128, HC], F32, tag="big")
                        nc.tensor.matmul(zps[:], wcb[:, c, :], rw[:], start=True, stop=False)
                        nc.tensor.matmul(zps[:], idb[:], gsl, start=False, stop=True)
                        st = pool.tile([128, HC], BF16, tag="st", bufs=3)
                        nc.scalar.activation(st[:], zps[:], AF.Prelu, alpha=0.2)
                        tt = pool.tile([128, HC], BF16, tag="tt", bufs=3)
                        for h in range(H):
                            nc.vector.scalar_tensor_tensor(
                                out=tt[:, h * C:(h + 1) * C],
                                in0=st[:, h * C:(h + 1) * C], scalar=1.0,
                                in1=attb[0:128, h * C:(h + 1) * C],
                                op0=ALU.mult, op1=ALU.mult,
                                accum_out=lgB[:, c * H + h:c * H + h + 1])
                    aBf = pool.tile([128, nch * H], F32, tag="aBf")
                    nc.scalar.activation(aBf[:], lgB[:], AF.Exp)
                    aBb = pool.tile([128, nch * H], BF16, tag="aBb")
                    nc.scalar.copy(out=aBb[:], in_=aBf[:])
                    for c in range(nch):
                        gsl = gt[:, c * HC:(c + 1) * HC]
                        sreqv = pool.tile([128, HC], BF16, tag="sr", bufs=3)
                        for h in range(H):
                            nc.vector.tensor_scalar(
                                out=sreqv[:, h * C:(h + 1) * C],
                                in0=gsl[:, h * C:(h + 1) * C],
                                scalar1=aBf[:, c * H + h:c * H + h + 1],
                                scalar2=None, op0=ALU.mult)
                        nc.tensor.matmul(out_ps[:], pcb[:, c, 0:125], sreqv[:],
                                         start=(c == 0), stop=(c == nch - 1))
                        nc.tensor.matmul(den_ps[:, 0:H], pcb[:, c, 0:125], aBb[:, c * H:(c + 1) * H],
                                         start=(c == 0), stop=(c == nch - 1))
                    cpos += nch

                    rden = pool.tile([125, H], F32, tag="rden")
                    nc.vector.reciprocal(out=rden[:], in_=den_ps[:, 0:H])
                    outp = pool.tile([125, HC], F32, tag="outp")
                    nc.vector.tensor_tensor(
                        out=outp[:].rearrange("p (h c) -> p h c", h=H),
                        in0=out_ps[:].rearrange("p (h c) -> p h c", h=H),
                        in1=rden[:].unsqueeze(2).broadcast_to([125, H, C]), op=ALU.mult)
                    rsum = pool.tile([125, 1], F32, tag="rsum")
                    opre = pool.tile([125, HC], BF16, tag="opre")
                    nc.vector.scalar_tensor_tensor(out=opre[:], in0=outp[:], scalar=1.0,
                                                   in1=bob[0:125, 0:HC], op0=ALU.mult, op1=ALU.add,
                                                   accum_out=rsum[:])
                    sq = pool.tile([125, HC], BF16, tag="sq")
                    sqa = pool.tile([125, 1], F32, tag="sqa")
                    nc.scalar.activation(sq[:], opre[:], AF.Square, accum_out=sqa[:])
                    nc.vector.tensor_tensor(out=statsum[0:125, 0:1], in0=statsum[0:125, 0:1],
                                            in1=rsum[:], op=ALU.add)
                    nc.vector.tensor_tensor(out=statsum[0:125, 1:2], in0=statsum[0:125, 1:2],
                                            in1=sqa[:], op=ALU.add)
                    nc.sync.dma_start(out=op_tbl[b * BLK:b * BLK + 125, 0:HC], in_=opre[:])

                # LN stats
                ones_t = pool.tile([128, 1], F32, tag="ones")
                nc.vector.memset(ones_t[:], 1.0)
                tot_ps = pp.tile([128, 128], F32, tag="tr")
                nc.tensor.matmul(tot_ps[0:1, 0:2], ones_t[:], statsum[:], start=True, stop=True)
                tot_sb = pool.tile([1, 2], F32, tag="tot")
                nc.scalar.copy(out=tot_sb[:], in_=tot_ps[0:1, 0:2])
                nc.sync.dma_start(out=st_in[:], in_=tot_sb[:])
                nc.gpsimd.collective_compute("AllReduce", ALU.add, replica_groups=RG,
                                             ins=[st_in[:]], outs=[st_out[:]])
                glob = pool.tile([1, 2], F32, tag="glob")
                nc.sync.dma_start(out=glob[:], in_=st_out[:])
                mm = pool.tile([1, 8], F32, tag="mmt")
                nc.vector.tensor_scalar(out=mm[:, 0:2], in0=glob[:], scalar1=d["inv_kn"],
                                        scalar2=None, op0=ALU.mult)
                nc.vector.tensor_tensor(out=mm[:, 2:3], in0=mm[:, 0:1], in1=mm[:, 0:1], op=ALU.mult)
                nc.vector.tensor_tensor(out=mm[:, 3:4], in0=mm[:, 1:2], in1=mm[:, 2:3], op=ALU.subtract)
                nc.vector.tensor_scalar(out=mm[:, 4:5], in0=mm[:, 3:4], scalar1=0.0,
                                        scalar2=None, op0=ALU.max)
                nc.scalar.activation(mm[:, 5:6], mm[:, 4:5], AF.Sqrt)
                nc.vector.tensor_scalar(out=mm[:, 5:6], in0=mm[:, 5:6], scalar1=1e-5,
                                        scalar2=None, op0=ALU.add)
                murs = pool.tile([1, 2], F32, tag="murs")
                nc.vector.reciprocal(out=murs[:, 1:2], in_=mm[:, 5:6])
                nc.vector.tensor_scalar(out=murs[:, 0:1], in0=mm[:, 0:1], scalar1=-1.0,
                                        scalar2=None, op0=ALU.mult)
                on1 = pool.tile([1, 128], F32, tag="on1")
                nc.vector.memset(on1[:], 1.0)
                rep_ps = pp.tile([128, 128], F32, tag="tr")
                nc.tensor.matmul(rep_ps[:, 0:2], on1[:], murs[:], start=True, stop=True)
                repc = pool.tile([128, 2], F32, tag="repc")
                nc.scalar.copy(out=repc[:], in_=rep_ps[:, 0:2])
                lnwr = pool.tile([128, HC], F32, tag="lnwr")
                nc.sync.dma_start(out=lnwr[:], in_=d["lnw_rep"][:])
                lnbr = pool.tile([128, HC], F32, tag="lnbr")
                nc.sync.dma_start(out=lnbr[:], in_=d["lnb_rep"][:])
                srep = apool.tile([128, HCm], F32, tag="srep")
                nc.vector.tensor_scalar(out=srep[0:128, 0:HC], in0=lnwr[:], scalar1=repc[:, 1:2],
                                        scalar2=None, op0=ALU.mult)
                brep = apool.tile([128, HCm], F32, tag="brep")
                nc.vector.scalar_tensor_tensor(out=brep[0:128, 0:HC], in0=srep[0:128, 0:HC],
                                               scalar=repc[:, 0:1], in1=lnbr[:],
                                               op0=ALU.mult, op1=ALU.add)

                # next-layer dense prep
                last = li == len(L) - 1
                if not last:
                    dn = L[li + 1]
                    ktn = dn["din"] // 128
                    wl_sb = apool.tile([128, ktn, HCm], BF16, tag="dWl")
                    nc.sync.dma_start(out=wl_sb[:, :, 0:dn["HC"]],
                                      in_=dn["Wl"][:].rearrange("(k p) c -> p k c", k=ktn))
                    wr_sb = apool.tile([128, ktn, HCm], BF16, tag="dWr")
                    nc.sync.dma_start(out=wr_sb[:, :, 0:dn["HC"]],
                                      in_=dn["Wr"][:].rearrange("(k p) c -> p k c", k=ktn))
                    brtn = apool.tile([128, HCm], F32, tag="dBr")
                    nc.sync.dma_start(out=brtn[0:128, 0:dn["HC"]], in_=dn["brbl_rep"][:])
                    eml = DenseEmitter(li + 1, dn, wl_sb, lambda n: (ag_in[li + 1], n * 128))
                    ndone = 0

                # pass B: LN + ELU -> hT; interleave next-layer Wl dense
                for b in range(nblk):
                    op_in = pool.tile([125, HC], BF16, tag="opin")
                    nc.sync.dma_start(out=op_in[:], in_=op_tbl[b * BLK:b * BLK + 125, 0:HC])
                    yv = pool.tile([125, HC], F32, tag="yv")
                    nc.vector.tensor_tensor(out=yv[:], in0=op_in[:], in1=srep[0:125, 0:HC], op=ALU.mult)
                    nc.vector.tensor_tensor(out=yv[:], in0=yv[:], in1=brep[0:125, 0:HC], op=ALU.add)
                    tmin = pool.tile([125, HC], F32, tag="tmin")
                    nc.vector.tensor_scalar(out=tmin[:], in0=yv[:], scalar1=0.0,
                                            scalar2=None, op0=ALU.min)
                    ev = pool.tile([125, HC], F32, tag="ev")
                    nc.scalar.activation(ev[:], tmin[:], AF.Exp)
                    rv = pool.tile([125, HC], F32, tag="rv")
                    nc.scalar.activation(rv[:], yv[:], AF.Relu)
                    hv = pool.tile([125, HC], BF16, tag="hv")
                    nc.vector.scalar_tensor_tensor(out=hv[:], in0=ev[:], scalar=-1.0,
                                                   in1=rv[:], op0=ALU.add, op1=ALU.add)
                    hTb = pool.tile([128, HC // 128, 125], BF16, tag="hTb")
                    for s in range(HC // 128):
                        tps = pp.tile([128, 128], BF16, tag="trb")
                        nc.tensor.matmul(tps[:, 0:125], hv[:, s * 128:(s + 1) * 128],
                                         idb[0:125, 0:125], is_transpose=True, start=True, stop=True)
                        nc.scalar.copy(out=hTb[:, s, :], in_=tps[:, 0:125])
                    if last:
                        half = 0 if (b + 1) * BLK <= HB else 1
                        coff = b * BLK - (0 if half == 0 else HB)
                        nc.sync.dma_start(out=hT3h[half][:, coff:coff + 125], in_=hTb[:, 0, :])
                        if (b + 1) * BLK == HB:
                            nc.gpsimd.collective_compute(
                                "AllGather", ALU.bypass, replica_groups=RG,
                                ins=[hT3h[0][:]], outs=[h3T_agh[0][:]])
                    else:
                        nc.sync.dma_start(
                            out=hT_tbl[li][:, b * BLK:b * BLK + 125]
                            .rearrange("(s p) j -> p s j", s=HC // 128),
                            in_=hTb[:])
                        # emit next-layer Wl tiles whose hT columns are ready
                        nrdy = min(((b + 1) * BLK) // 128, ntile - 1)
                        while ndone < nrdy:
                            eml.emit(ndone)
                            ndone += 1
                if last:
                    if tpad > shard:
                        nc.sync.dma_start(out=hT3h[1][:, shard - HB:tpad - HB],
                                          in_=zbf_t[0:128, 0:tpad - shard])
                    nc.gpsimd.collective_compute(
                        "AllGather", ALU.bypass, replica_groups=RG,
                        ins=[hT3h[1][:]], outs=[h3T_agh[1][:]])
                else:
                    if tpad > shard:
                        for s in range(HC // 128):
                            nc.sync.dma_start(out=hT_tbl[li][s * 128:(s + 1) * 128, shard:tpad],
                                              in_=zbf_t[0:128, 0:tpad - shard])
                    while ndone < ntile:
                        eml.emit(ndone)
                        ndone += 1
                    eml.flush()
                    nc.gpsimd.collective_compute(
                        "AllGather", ALU.bypass, replica_groups=RG,
                        ins=[ag_in[li + 1][:]], outs=[xl_tbl[li + 1][:]])
                    emr = DenseEmitter(li + 1, dn, wr_sb, lambda n: (xr_tbl[li + 1], n * 128),
                                       brt=brtn)
                    for n in range(ntile):
                        emr.emit(n)
                    emr.flush()

            # ---------------- pooling ----------------
            pieces2 = []
            for (g, k, a, b_) in pieces:
                if a < HB:
                    pieces2.append((g, k, 0, a, min(b_, HB)))
                if b_ > HB:
                    pieces2.append((g, k, 1, max(a, HB) - HB, b_ - HB))
            msum = apool.tile([C, GP], F32, tag="msum")
            nc.vector.memset(msum[:], 0.0)
            mmax = apool.tile([C, GP], F32, tag="mmax")
            nc.vector.memset(mmax[:], -3.0e38)
            for half in (0, 1):
                for k in range(NCORE):
                    kp = [p for p in pieces2 if p[1] == k and p[2] == half]
                    if not kp:
                        continue
                    W = HB if half == 0 else tpad - HB
                    hbt = pool.tile([C, W], BF16, tag="hb")
                    nc.sync.dma_start(out=hbt[:], in_=h3T_agh[half][k * C:(k + 1) * C, :])
                    for (g, _, _, a, b_) in kp:
                        red = pool.tile([C, 2], F32, tag="red")
                        nc.vector.tensor_reduce(out=red[:, 0:1], in_=hbt[:, a:b_], axis=AX.XYZW, op=ALU.add)
                        nc.vector.tensor_reduce(out=red[:, 1:2], in_=hbt[:, a:b_], axis=AX.XYZW, op=ALU.max)
                        nc.vector.tensor_tensor(out=msum[:, g:g + 1], in0=msum[:, g:g + 1],
                                                in1=red[:, 0:1], op=ALU.add)
                        nc.vector.tensor_tensor(out=mmax[:, g:g + 1], in0=mmax[:, g:g + 1],
                                                in1=red[:, 1:2], op=ALU.max)
            for part, scale_t, off in ((msum, ginv_t, 0), (mmax, gmask_t, C)):
                for g0 in range(0, GP, 128):
                    gw = min(128, GP - g0)
                    tps = pp.tile([128, 128], F32, tag="tr")
                    nc.tensor.matmul(tps[0:gw, 0:C], part[:, g0:g0 + gw], idf[:],
                                     is_transpose=True, start=True, stop=True)
                    sc = pool.tile([128, 1], F32, tag="sc")
                    nc.sync.dma_start(out=sc[0:gw, :], in_=scale_t[g0:g0 + gw, :])
                    yt = pool.tile([128, C], F32, tag="yt")
                    nc.vector.tensor_scalar(out=yt[0:gw, :], in0=tps[0:gw, 0:C],
                                            scalar1=sc[0:gw, :], scalar2=None, op0=ALU.mult)
                    lo_g, hi_g = g0, min(G, g0 + gw)
                    if hi_g > lo_g:
                        nc.sync.dma_start(out=y_out[lo_g:hi_g, off:off + C],
                                          in_=yt[0:hi_g - lo_g, :])

    nc.finalize()
    return nc, shared


def kernel(**inputs):
    x = np.asarray(inputs["x"], np.float32)
    edge_index = np.asarray(inputs["edge_index"])
    edge_attr = np.asarray(inputs["edge_attr"], np.float32)
    batch = np.asarray(inputs["batch"])
    G = 64
    meta, per_core, pieces, ginv, gmask = _preprocess(x, edge_index, edge_attr, batch, G)
    params = {k: np.asarray(v, np.float32) for k, v in inputs.items()
              if k not in ("x", "edge_index", "edge_attr", "batch")}
    nc, shared = _build(meta, params, pieces, ginv, gmask)
    in_maps = []
    for k in range(NCORE):
        m = dict(shared)
        for name, arr in per_core[k].items():
            m[name] = np.ascontiguousarray(arr)
        in_maps.append(m)
    import os
    trace = bool(os.environ.get("KBENCH_TRACE"))
    res = run_bass_kernel_spmd(nc, in_maps, core_ids=list(range(NCORE)), trace=trace)
    global LAST_EXEC_NS, LAST_RES
    LAST_EXEC_NS = res.exec_time_ns
    LAST_RES = res
    return np.asarray(res.results[0]["y"], np.float32)


def benchmark(n_iters=3, **inputs):
    """Run once for correctness, then time pure device execution of the
    compiled SPMD program with device-resident inputs."""
    import time
    import jax
    from jax.sharding import Mesh, PartitionSpec, NamedSharding
    from jax.experimental.shard_map import shard_map
    from concourse import bass2jax

    x = np.asarray(inputs["x"], np.float32)
    edge_index = np.asarray(inputs["edge_index"])
    edge_attr = np.asarray(inputs["edge_attr"], np.float32)
    batch = np.asarray(inputs["batch"])
    meta, per_core, pieces, ginv, gmask = _preprocess(x, edge_index, edge_attr, batch, 64)
    params = {k: np.asarray(v, np.float32) for k, v in inputs.items()
              if k not in ("x", "edge_index", "edge_attr", "batch")}
    nc, shared = _build(meta, params, pieces, ginv, gmask)
    in_maps = []
    for k in range(NCORE):
        m = dict(shared)
        for name, arr in per_core[k].items():
            m[name] = np.ascontiguousarray(arr)
        in_maps.append(m)

    bass2jax.install_neuronx_cc_hook()
    n_cores = NCORE
    in_names, out_names, out_avals, zero_outs = [], [], [], []
    partition_name = nc.partition_id_tensor.name if nc.partition_id_tensor else None
    for alloc in nc.m.functions[0].allocations:
        if not isinstance(alloc, mybir.MemoryLocationSet):
            continue
        name = alloc.memorylocations[0].name
        if alloc.kind == "ExternalInput":
            if name != partition_name:
                in_names.append(name)
        elif alloc.kind == "ExternalOutput":
            out_names.append(name)
            shape = tuple(alloc.tensor_shape)
            dtype = mybir.dt.np(alloc.dtype)
            out_avals.append(jax.core.ShapedArray(shape, dtype))
            zero_outs.append(np.zeros(shape, dtype))
    n_params = len(in_names)
    all_in = list(in_names) + list(out_names)
    if partition_name is not None:
        all_in.append(partition_name)

    def _body(*args):
        ops = list(args)
        if partition_name is not None:
            ops.append(bass2jax.partition_id_tensor())
        return tuple(bass2jax._bass_exec_p.bind(
            *ops, out_avals=tuple(out_avals), in_names=tuple(all_in),
            out_names=tuple(out_names), lowering_input_output_aliases=(),
            sim_require_finite=True, sim_require_nnan=True, nc=nc))

    devices = jax.devices()[:n_cores]
    mesh = Mesh(np.asarray(devices), ("core",))
    nin = n_params + len(zero_outs)
    sharded = jax.jit(shard_map(_body, mesh=mesh,
                                in_specs=(PartitionSpec("core"),) * nin,
                                out_specs=(PartitionSpec("core"),) * len(out_names),
                                check_rep=False),
                      keep_unused=True)
    sh = NamedSharding(mesh, PartitionSpec("core"))
    concat_in = [jax.device_put(
        np.concatenate([np.asarray(in_maps[c][nm]) for c in range(n_cores)], axis=0), sh)
        for nm in in_names]
    concat_zeros = [jax.device_put(
        np.zeros((n_cores * z.shape[0], *z.shape[1:]), z.dtype), sh) for z in zero_outs]
    for a in concat_in:
        a.block_until_ready()
    outs = sharded(*concat_in, *concat_zeros)
    jax.block_until_ready(outs)
    y = np.asarray(outs[out_names.index("y")]).reshape(n_cores, *out_avals[out_names.index("y")].shape)[0]
    times = []
    for _ in range(n_iters):
        t0 = time.time()
        outs = sharded(*concat_in, *concat_zeros)
        jax.block_until_ready(outs)
        times.append(time.time() - t0)
    return y, min(times)

